# revision 9
# baseline (speedup 1.0000x reference)
"""ConvCapsule Trainium2 kernel.

Full inputs -> 8-way batch-parallel (over output batch b) -> full output.

Math (per core, b = core id):
  img j in 0..7:  votes[j] = conv3x3_SAME(x[j, :, :, b, :], W)  -> [32,32,256]
  preact1 = (1/16) * sum_j votes[j] + bias          (softmax of zero logits = 1/16)
  act1    = squash(preact1)   [squash over dc groups of 16]
  logits[j, s, nc] = sum_dc votes[j][s, nc, dc] * act1[s, nc, dc]
  route   = softmax(logits over nc)
  preact2 = sum_j route[j] * votes[j] + bias
  out     = squash(preact2)

Device mapping:
  - conv as 2 accumulated matmuls (K=96/97 + K=48) per 128-pixel chunk per image,
    using a host-built im2col tensor S (6 w-shifted channel groups, zero pad,
    ones row for fused bias).
  - preact1 via duplicate matmuls with W/16-scaled weights accumulating in PSUM.
  - routing on DVE/ACT/GPSIMD; squash factor applied after the grouped reduce
    (linearity), fused route construction via scalar_tensor_tensor.
"""

import numpy as np

import concourse.bacc as bacc
import concourse.tile as tile
from concourse import mybir
from concourse import bass_utils

F32 = mybir.dt.float32
AF = mybir.ActivationFunctionType
OP = mybir.AluOpType

B, H, W_, NIN, DIN = 8, 32, 32, 8, 16
NC, DC = 16, 16
O = NC * DC           # 256 out channels
SF = 34 * 32 + 64     # S free dim: 34 zero-padded rows of 32, + tail for +2-row reads
EPS = 1e-9
NCHUNK = 8            # spatial chunks of 128 pixels (4 rows)
NCORES = 8
GPSIMD_DMULTS = 0     # how many of the 8 route*votes products go to GPSIMD
GPSIMD_BMULT = False   # B-product on gpsimd

_CACHE = {}


def build_module():
    nc = bacc.Bacc("TRN2", target_bir_lowering=False, debug=False)

    s_all = nc.dram_tensor("s_all", [NIN, 97, SF], F32, kind="ExternalInput")
    wc96 = nc.dram_tensor("wc96", [96, O], F32, kind="ExternalInput")
    wc48 = nc.dram_tensor("wc48", [48, O], F32, kind="ExternalInput")
    wc96s = nc.dram_tensor("wc96s", [97, O], F32, kind="ExternalInput")
    wc48s = nc.dram_tensor("wc48s", [48, O], F32, kind="ExternalInput")
    brep = nc.dram_tensor("brep", [128, O], F32, kind="ExternalInput")
    out = nc.dram_tensor("out", [H, W_, O], F32, kind="ExternalOutput")

    with tile.TileContext(nc) as tc:
        with (
            tc.tile_pool(name="const", bufs=1) as constp,
            tc.tile_pool(name="simg", bufs=1) as sp,
            tc.tile_pool(name="psum", bufs=1, space="PSUM") as pp,
            tc.tile_pool(name="work", bufs=2) as wp,
            tc.tile_pool(name="small", bufs=2) as smp,
        ):
            # ---- persistent loads ----
            w96 = constp.tile([96, O], F32)
            w48 = constp.tile([48, O], F32)
            w96s = constp.tile([97, O], F32)
            w48s = constp.tile([48, O], F32)
            bias = constp.tile([128, O], F32)
            nc.sync.dma_start(w96[:], wc96.ap())
            nc.sync.dma_start(w48[:], wc48.ap())
            nc.sync.dma_start(w96s[:], wc96s.ap())
            nc.sync.dma_start(w48s[:], wc48s.ap())
            nc.sync.dma_start(bias[:], brep.ap())

            s_tiles = []
            for j in range(NIN):
                st = sp.tile([97, SF], F32, name=f"s{j}")
                nc.sync.dma_start(st[:], s_all.ap()[j])
                s_tiles.append(st)

            for c in range(NCHUNK):
                h0 = 4 * c
                # ---------------- conv ----------------
                ps_votes = pp.tile([128, NIN * O], F32, tag="psv", bufs=1)
                ps_pre1 = pp.tile([128, O], F32, tag="psp", bufs=1)
                p0 = (h0 + 1) * 32
                for j in range(NIN):
                    st = s_tiles[j]
                    l96 = st[0:96, p0:p0 + 128]
                    l97 = st[0:97, p0:p0 + 128]
                    l48 = st[0:48, p0 + 64:p0 + 192]
                    vslice = ps_votes[:, j * O:(j + 1) * O]
                    nc.tensor.matmul(vslice, l96, w96[:], start=True, stop=False,
                                     skip_group_check=True)
                    if j == 0:
                        nc.tensor.matmul(ps_pre1[:], l97, w96s[:],
                                         start=True, stop=False,
                                         skip_group_check=True)
                    else:
                        nc.tensor.matmul(ps_pre1[:], l96, w96s[0:96],
                                         start=False, stop=False,
                                         skip_group_check=True)
                    nc.tensor.matmul(vslice, l48, w48[:], start=False, stop=True,
                                     skip_group_check=True)
                    nc.tensor.matmul(ps_pre1[:], l48[0:48], w48s[:],
                                     start=False, stop=(j == NIN - 1),
                                     skip_group_check=True)

                # ---------------- evict ----------------
                votes = wp.tile([128, NIN * O], F32, tag="votes")
                pre1 = smp.tile([128, O], F32, tag="pre1")
                nc.scalar.copy(votes[:], ps_votes[:])
                nc.scalar.copy(pre1[:], ps_pre1[:])

                # ---------------- squash factor f1 from preact1 ----------------
                sqel1 = smp.tile([128, O], F32, tag="sqel1")
                nc.scalar.square(sqel1[:], pre1[:])
                sq1 = smp.tile([128, NC], F32, tag="sq1")
                nc.vector.reduce_sum(
                    sq1[:], sqel1[:].rearrange("p (n d) -> p n d", d=DC),
                    axis=mybir.AxisListType.X)
                f1 = _squash_factor(nc, smp, sq1, "1")

                # ---------------- logits ----------------
                pall = wp.tile([128, NIN * O], F32, tag="pall")
                v3 = votes[:].rearrange("p (j o) -> p j o", j=NIN)
                p1b = pre1[:].unsqueeze(1).broadcast_to([128, NIN, O])
                eng_b = nc.gpsimd if GPSIMD_BMULT else nc.vector
                eng_b.tensor_tensor(
                    pall[:].rearrange("p (j o) -> p j o", j=NIN), v3, p1b, op=OP.mult)
                lg = smp.tile([128, NIN * NC], F32, tag="lg")
                nc.vector.reduce_sum(
                    lg[:], pall[:].rearrange("p (j n d) -> p j n d", n=NC, d=DC),
                    axis=mybir.AxisListType.X)
                logits = smp.tile([128, NIN * NC], F32, tag="logits")
                f1b = f1[:].unsqueeze(1).broadcast_to([128, NIN, NC])
                nc.vector.tensor_tensor(
                    logits[:].rearrange("p (j n) -> p j n", j=NIN),
                    lg[:].rearrange("p (j n) -> p j n", j=NIN), f1b, op=OP.mult)

                # ---------------- softmax over nc ----------------
                ee = smp.tile([128, NIN * NC], F32, tag="ee")
                nc.scalar.activation(ee[:], logits[:], AF.Exp)
                den = smp.tile([128, NIN], F32, tag="den")
                nc.vector.reduce_sum(
                    den[:], ee[:].rearrange("p (j n) -> p j n", j=NIN),
                    axis=mybir.AxisListType.X)
                rcp = smp.tile([128, NIN], F32, tag="rcp")
                nc.vector.reciprocal(rcp[:], den[:])

                # ---------------- preact2 = sum_j route*votes + b ----------------
                route = smp.tile([128, NIN * NC], F32, tag="route")
                rcpb = rcp[:].unsqueeze(2).broadcast_to([128, NIN, NC])
                nc.vector.tensor_tensor(
                    route[:].rearrange("p (j n) -> p j n", j=NIN),
                    ee[:].rearrange("p (j n) -> p j n", j=NIN), rcpb, op=OP.mult)
                p2 = wp.tile([128, NIN * O], F32, tag="p2")
                for j in range(NIN):
                    rj = route[:, j * NC:(j + 1) * NC]
                    rjb = rj.unsqueeze(2).broadcast_to([128, NC, DC])
                    eng = nc.gpsimd if j < GPSIMD_DMULTS else nc.vector
                    eng.tensor_tensor(
                        p2[:, j * O:(j + 1) * O].rearrange("p (n d) -> p n d", n=NC),
                        votes[:, j * O:(j + 1) * O].rearrange("p (n d) -> p n d", n=NC),
                        rjb, op=OP.mult)
                pre2 = smp.tile([128, O], F32, tag="pre2")
                nc.vector.reduce_sum(
                    pre2[:],
                    p2[:].rearrange("p (j n d) -> p n d j", j=NIN, n=NC),
                    axis=mybir.AxisListType.X)
                pre2b = smp.tile([128, O], F32, tag="pre2b")
                nc.vector.tensor_tensor(pre2b[:], pre2[:], bias[:], op=OP.add)

                # ---------------- final squash ----------------
                sqel2 = smp.tile([128, O], F32, tag="sqel2")
                nc.scalar.square(sqel2[:], pre2b[:])
                sq2 = smp.tile([128, NC], F32, tag="sq2")
                nc.vector.reduce_sum(
                    sq2[:], sqel2[:].rearrange("p (n d) -> p n d", d=DC),
                    axis=mybir.AxisListType.X)
                f2 = _squash_factor(nc, smp, sq2, "2")
                act2 = wp.tile([128, O], F32, tag="act2")
                f2b = f2[:].unsqueeze(2).broadcast_to([128, NC, DC])
                nc.vector.tensor_tensor(
                    act2[:].rearrange("p (n d) -> p n d", n=NC),
                    pre2b[:].rearrange("p (n d) -> p n d", n=NC), f2b, op=OP.mult)

                nc.sync.dma_start(
                    out.ap().rearrange("h w o -> (h w) o")[c * 128:(c + 1) * 128],
                    act2[:])

    nc.compile()
    return nc


def _squash_factor(nc, pool, sq, tag):
    """f = sq / ((1+sq) * sqrt(sq+EPS)), shape [128, NC]."""
    sqe = pool.tile([128, NC], F32, name=f"sqe{tag}", tag=f"sqe{tag}")
    nc.vector.tensor_scalar_add(sqe[:], sq[:], EPS)
    rt = pool.tile([128, NC], F32, name=f"rt{tag}", tag=f"rt{tag}")
    nc.scalar.activation(rt[:], sqe[:], AF.Sqrt)
    u = pool.tile([128, NC], F32, name=f"u{tag}", tag=f"u{tag}")
    nc.vector.tensor_scalar_add(u[:], sq[:], 1.0)
    w = pool.tile([128, NC], F32, name=f"w{tag}", tag=f"w{tag}")
    nc.vector.tensor_tensor(w[:], u[:], rt[:], op=OP.mult)
    vr = pool.tile([128, NC], F32, name=f"vr{tag}", tag=f"vr{tag}")
    nc.vector.reciprocal(vr[:], w[:])
    f = pool.tile([128, NC], F32, name=f"f{tag}", tag=f"f{tag}")
    nc.vector.tensor_tensor(f[:], sq[:], vr[:], op=OP.mult)
    return f


def make_inputs(x, W, b):
    """Host-side shard + layout prep. Core i gets x[:, :, :, i, :] im2col'd."""
    x = np.asarray(x, dtype=np.float32)
    W = np.asarray(W, dtype=np.float32)
    b = np.asarray(b, dtype=np.float32)

    # weight tables, shared by all cores
    wc96 = np.zeros((96, O), np.float32)
    wc48 = np.zeros((48, O), np.float32)
    for g in range(6):
        kh, kw = (0, g) if g < 3 else (1, g - 3)
        wc96[16 * g:16 * g + 16] = W[kh, kw]
    for g in range(3):
        wc48[16 * g:16 * g + 16] = W[2, g]
    bflat = b.reshape(O)
    wc96s = np.concatenate([wc96 / 16.0, bflat[None, :]], axis=0).astype(np.float32)
    wc48s = (wc48 / 16.0).astype(np.float32)
    brep = np.broadcast_to(bflat, (128, O)).copy()

    shifts = [(-1, -1), (-1, 0), (-1, 1), (0, -1), (0, 0), (0, 1)]
    in_maps = []
    for core in range(NCORES):
        xs = np.transpose(x[:, :, :, core, :], (0, 3, 1, 2))  # [8, 16, 32, 32]
        s = np.zeros((NIN, 97, SF), np.float32)
        s[:, 96, :] = 1.0
        for g, (dh, dw) in enumerate(shifts):
            G = np.zeros((NIN, DIN, 34, 32), np.float32)
            rlo, rhi = max(0, 1 - dh), min(34, 33 - dh)
            wlo, whi = max(0, -dw), min(32, 32 - dw)
            G[:, :, rlo:rhi, wlo:whi] = xs[:, :, rlo - 1 + dh:rhi - 1 + dh, wlo + dw:whi + dw]
            s[:, 16 * g:16 * g + 16, :34 * 32] = G.reshape(NIN, DIN, 34 * 32)
        in_maps.append({
            "s_all": s, "wc96": wc96, "wc48": wc48,
            "wc96s": wc96s, "wc48s": wc48s, "brep": brep,
        })
    return in_maps


def kernel(x, W, b):
    if "nc" not in _CACHE:
        _CACHE["nc"] = build_module()
    nc = _CACHE["nc"]
    in_maps = make_inputs(x, W, b)
    res = bass_utils.run_bass_kernel_spmd(nc, in_maps, core_ids=list(range(NCORES)))
    outs = [res.results[c]["out"].reshape(H, W_, NC, DC) for c in range(NCORES)]
    return np.stack(outs, axis=0)


# revision 10
# speedup vs baseline: 1.0322x; 1.0322x over previous
"""ConvCapsule Trainium2 kernel.

Full inputs -> 8-way batch-parallel (over output batch b) -> full output.

Math (per core, b = core id):
  img j in 0..7:  votes[j] = conv3x3_SAME(x[j, :, :, b, :], W)  -> [32,32,256]
  preact1 = (1/16) * sum_j votes[j] + bias          (softmax of zero logits = 1/16)
  act1    = squash(preact1)   [squash over dc groups of 16]
  logits[j, s, nc] = sum_dc votes[j][s, nc, dc] * act1[s, nc, dc]
  route   = softmax(logits over nc)
  preact2 = sum_j route[j] * votes[j] + bias
  out     = squash(preact2)

Device mapping:
  - conv as 2 accumulated matmuls (K=96/97 + K=48) per 128-pixel chunk per image,
    using a host-built im2col tensor S (6 w-shifted channel groups, zero pad,
    ones row for fused bias).
  - preact1 via duplicate matmuls with W/16-scaled weights accumulating in PSUM.
  - routing on DVE/ACT/GPSIMD; squash factor applied after the grouped reduce
    (linearity), fused route construction via scalar_tensor_tensor.
"""

import numpy as np

import concourse.bacc as bacc
import concourse.tile as tile
from concourse import mybir
from concourse import bass_utils

F32 = mybir.dt.float32
AF = mybir.ActivationFunctionType
OP = mybir.AluOpType

B, H, W_, NIN, DIN = 8, 32, 32, 8, 16
NC, DC = 16, 16
O = NC * DC           # 256 out channels
SF = 34 * 32 + 64     # S free dim: 34 zero-padded rows of 32, + tail for +2-row reads
EPS = 1e-9
NCHUNK = 8            # spatial chunks of 128 pixels (4 rows)
NCORES = 8
GPSIMD_DMULTS = 4     # how many of the 8 route*votes products go to GPSIMD
GPSIMD_BMULT = True   # B-product on gpsimd

_CACHE = {}


def build_module():
    nc = bacc.Bacc("TRN2", target_bir_lowering=False, debug=False)

    s_all = nc.dram_tensor("s_all", [NIN, 97, SF], F32, kind="ExternalInput")
    wc96 = nc.dram_tensor("wc96", [96, O], F32, kind="ExternalInput")
    wc48 = nc.dram_tensor("wc48", [48, O], F32, kind="ExternalInput")
    wc96s = nc.dram_tensor("wc96s", [97, O], F32, kind="ExternalInput")
    wc48s = nc.dram_tensor("wc48s", [48, O], F32, kind="ExternalInput")
    brep = nc.dram_tensor("brep", [128, O], F32, kind="ExternalInput")
    out = nc.dram_tensor("out", [H, W_, O], F32, kind="ExternalOutput")

    with tile.TileContext(nc) as tc:
        with (
            tc.tile_pool(name="const", bufs=1) as constp,
            tc.tile_pool(name="simg", bufs=1) as sp,
            tc.tile_pool(name="psum", bufs=1, space="PSUM") as pp,
            tc.tile_pool(name="work", bufs=2) as wp,
            tc.tile_pool(name="small", bufs=2) as smp,
        ):
            # ---- persistent loads ----
            w96 = constp.tile([96, O], F32)
            w48 = constp.tile([48, O], F32)
            w96s = constp.tile([97, O], F32)
            w48s = constp.tile([48, O], F32)
            bias = constp.tile([128, O], F32)
            nc.sync.dma_start(w96[:], wc96.ap())
            nc.sync.dma_start(w48[:], wc48.ap())
            nc.sync.dma_start(w96s[:], wc96s.ap())
            nc.sync.dma_start(w48s[:], wc48s.ap())
            nc.sync.dma_start(bias[:], brep.ap())

            s_tiles = []
            for j in range(NIN):
                st = sp.tile([97, SF], F32, name=f"s{j}")
                nc.sync.dma_start(st[:], s_all.ap()[j])
                s_tiles.append(st)

            for c in range(NCHUNK):
                h0 = 4 * c
                # ---------------- conv ----------------
                ps_votes = pp.tile([128, NIN * O], F32, tag="psv", bufs=1)
                ps_pre1 = pp.tile([128, O], F32, tag="psp", bufs=1)
                p0 = (h0 + 1) * 32
                for j in range(NIN):
                    st = s_tiles[j]
                    l96 = st[0:96, p0:p0 + 128]
                    l97 = st[0:97, p0:p0 + 128]
                    l48 = st[0:48, p0 + 64:p0 + 192]
                    vslice = ps_votes[:, j * O:(j + 1) * O]
                    nc.tensor.matmul(vslice, l96, w96[:], start=True, stop=False,
                                     skip_group_check=True)
                    if j == 0:
                        nc.tensor.matmul(ps_pre1[:], l97, w96s[:],
                                         start=True, stop=False,
                                         skip_group_check=True)
                    else:
                        nc.tensor.matmul(ps_pre1[:], l96, w96s[0:96],
                                         start=False, stop=False,
                                         skip_group_check=True)
                    nc.tensor.matmul(vslice, l48, w48[:], start=False, stop=True,
                                     skip_group_check=True)
                    nc.tensor.matmul(ps_pre1[:], l48[0:48], w48s[:],
                                     start=False, stop=(j == NIN - 1),
                                     skip_group_check=True)

                # ---------------- evict ----------------
                votes = wp.tile([128, NIN * O], F32, tag="votes")
                pre1 = smp.tile([128, O], F32, tag="pre1")
                nc.scalar.copy(votes[:], ps_votes[:])
                nc.scalar.copy(pre1[:], ps_pre1[:])

                # ---------------- squash factor f1 from preact1 ----------------
                sqel1 = smp.tile([128, O], F32, tag="sqel1")
                nc.scalar.square(sqel1[:], pre1[:])
                sq1 = smp.tile([128, NC], F32, tag="sq1")
                nc.vector.reduce_sum(
                    sq1[:], sqel1[:].rearrange("p (n d) -> p n d", d=DC),
                    axis=mybir.AxisListType.X)
                f1 = _squash_factor(nc, smp, sq1, "1")

                # ---------------- logits ----------------
                pall = wp.tile([128, NIN * O], F32, tag="pall")
                v3 = votes[:].rearrange("p (j o) -> p j o", j=NIN)
                p1b = pre1[:].unsqueeze(1).broadcast_to([128, NIN, O])
                eng_b = nc.gpsimd if GPSIMD_BMULT else nc.vector
                eng_b.tensor_tensor(
                    pall[:].rearrange("p (j o) -> p j o", j=NIN), v3, p1b, op=OP.mult)
                lg = smp.tile([128, NIN * NC], F32, tag="lg")
                nc.vector.reduce_sum(
                    lg[:], pall[:].rearrange("p (j n d) -> p j n d", n=NC, d=DC),
                    axis=mybir.AxisListType.X)
                logits = smp.tile([128, NIN * NC], F32, tag="logits")
                f1b = f1[:].unsqueeze(1).broadcast_to([128, NIN, NC])
                nc.vector.tensor_tensor(
                    logits[:].rearrange("p (j n) -> p j n", j=NIN),
                    lg[:].rearrange("p (j n) -> p j n", j=NIN), f1b, op=OP.mult)

                # ---------------- softmax over nc ----------------
                ee = smp.tile([128, NIN * NC], F32, tag="ee")
                nc.scalar.activation(ee[:], logits[:], AF.Exp)
                den = smp.tile([128, NIN], F32, tag="den")
                nc.vector.reduce_sum(
                    den[:], ee[:].rearrange("p (j n) -> p j n", j=NIN),
                    axis=mybir.AxisListType.X)
                rcp = smp.tile([128, NIN], F32, tag="rcp")
                nc.vector.reciprocal(rcp[:], den[:])

                # ---------------- preact2 = sum_j route*votes + b ----------------
                route = smp.tile([128, NIN * NC], F32, tag="route")
                rcpb = rcp[:].unsqueeze(2).broadcast_to([128, NIN, NC])
                nc.vector.tensor_tensor(
                    route[:].rearrange("p (j n) -> p j n", j=NIN),
                    ee[:].rearrange("p (j n) -> p j n", j=NIN), rcpb, op=OP.mult)
                p2 = wp.tile([128, NIN * O], F32, tag="p2")
                for j in range(NIN):
                    rj = route[:, j * NC:(j + 1) * NC]
                    rjb = rj.unsqueeze(2).broadcast_to([128, NC, DC])
                    eng = nc.gpsimd if j < GPSIMD_DMULTS else nc.vector
                    eng.tensor_tensor(
                        p2[:, j * O:(j + 1) * O].rearrange("p (n d) -> p n d", n=NC),
                        votes[:, j * O:(j + 1) * O].rearrange("p (n d) -> p n d", n=NC),
                        rjb, op=OP.mult)
                pre2 = smp.tile([128, O], F32, tag="pre2")
                nc.vector.reduce_sum(
                    pre2[:],
                    p2[:].rearrange("p (j n d) -> p n d j", j=NIN, n=NC),
                    axis=mybir.AxisListType.X)
                pre2b = smp.tile([128, O], F32, tag="pre2b")
                nc.vector.tensor_tensor(pre2b[:], pre2[:], bias[:], op=OP.add)

                # ---------------- final squash ----------------
                sqel2 = smp.tile([128, O], F32, tag="sqel2")
                nc.scalar.square(sqel2[:], pre2b[:])
                sq2 = smp.tile([128, NC], F32, tag="sq2")
                nc.vector.reduce_sum(
                    sq2[:], sqel2[:].rearrange("p (n d) -> p n d", d=DC),
                    axis=mybir.AxisListType.X)
                f2 = _squash_factor(nc, smp, sq2, "2")
                act2 = wp.tile([128, O], F32, tag="act2")
                f2b = f2[:].unsqueeze(2).broadcast_to([128, NC, DC])
                nc.vector.tensor_tensor(
                    act2[:].rearrange("p (n d) -> p n d", n=NC),
                    pre2b[:].rearrange("p (n d) -> p n d", n=NC), f2b, op=OP.mult)

                nc.sync.dma_start(
                    out.ap().rearrange("h w o -> (h w) o")[c * 128:(c + 1) * 128],
                    act2[:])

    nc.compile()
    return nc


def _squash_factor(nc, pool, sq, tag):
    """f = sq / ((1+sq) * sqrt(sq+EPS)), shape [128, NC]."""
    sqe = pool.tile([128, NC], F32, name=f"sqe{tag}", tag=f"sqe{tag}")
    nc.vector.tensor_scalar_add(sqe[:], sq[:], EPS)
    rt = pool.tile([128, NC], F32, name=f"rt{tag}", tag=f"rt{tag}")
    nc.scalar.activation(rt[:], sqe[:], AF.Sqrt)
    u = pool.tile([128, NC], F32, name=f"u{tag}", tag=f"u{tag}")
    nc.vector.tensor_scalar_add(u[:], sq[:], 1.0)
    w = pool.tile([128, NC], F32, name=f"w{tag}", tag=f"w{tag}")
    nc.vector.tensor_tensor(w[:], u[:], rt[:], op=OP.mult)
    vr = pool.tile([128, NC], F32, name=f"vr{tag}", tag=f"vr{tag}")
    nc.vector.reciprocal(vr[:], w[:])
    f = pool.tile([128, NC], F32, name=f"f{tag}", tag=f"f{tag}")
    nc.vector.tensor_tensor(f[:], sq[:], vr[:], op=OP.mult)
    return f


def make_inputs(x, W, b):
    """Host-side shard + layout prep. Core i gets x[:, :, :, i, :] im2col'd."""
    x = np.asarray(x, dtype=np.float32)
    W = np.asarray(W, dtype=np.float32)
    b = np.asarray(b, dtype=np.float32)

    # weight tables, shared by all cores
    wc96 = np.zeros((96, O), np.float32)
    wc48 = np.zeros((48, O), np.float32)
    for g in range(6):
        kh, kw = (0, g) if g < 3 else (1, g - 3)
        wc96[16 * g:16 * g + 16] = W[kh, kw]
    for g in range(3):
        wc48[16 * g:16 * g + 16] = W[2, g]
    bflat = b.reshape(O)
    wc96s = np.concatenate([wc96 / 16.0, bflat[None, :]], axis=0).astype(np.float32)
    wc48s = (wc48 / 16.0).astype(np.float32)
    brep = np.broadcast_to(bflat, (128, O)).copy()

    shifts = [(-1, -1), (-1, 0), (-1, 1), (0, -1), (0, 0), (0, 1)]
    in_maps = []
    for core in range(NCORES):
        xs = np.transpose(x[:, :, :, core, :], (0, 3, 1, 2))  # [8, 16, 32, 32]
        s = np.zeros((NIN, 97, SF), np.float32)
        s[:, 96, :] = 1.0
        for g, (dh, dw) in enumerate(shifts):
            G = np.zeros((NIN, DIN, 34, 32), np.float32)
            rlo, rhi = max(0, 1 - dh), min(34, 33 - dh)
            wlo, whi = max(0, -dw), min(32, 32 - dw)
            G[:, :, rlo:rhi, wlo:whi] = xs[:, :, rlo - 1 + dh:rhi - 1 + dh, wlo + dw:whi + dw]
            s[:, 16 * g:16 * g + 16, :34 * 32] = G.reshape(NIN, DIN, 34 * 32)
        in_maps.append({
            "s_all": s, "wc96": wc96, "wc48": wc48,
            "wc96s": wc96s, "wc48s": wc48s, "brep": brep,
        })
    return in_maps


def kernel(x, W, b):
    if "nc" not in _CACHE:
        _CACHE["nc"] = build_module()
    nc = _CACHE["nc"]
    in_maps = make_inputs(x, W, b)
    res = bass_utils.run_bass_kernel_spmd(nc, in_maps, core_ids=list(range(NCORES)))
    outs = [res.results[c]["out"].reshape(H, W_, NC, DC) for c in range(NCORES)]
    return np.stack(outs, axis=0)
